# revision 1
# baseline (speedup 1.0000x reference)
"""GCN layer (SpMM): out[r] = sum_{e: row(e)=r} val[e] * embeds[col(e)]
for N=100000 nodes, d=128, E=3200000 edges, distributed over 8 NeuronCores.

Sharding: 1D row partition — core k owns destination rows [k*12500, (k+1)*12500);
the embedding table is replicated. Per core the edges are bucketed by
(128-row output window, 25000-row column chunk); each bucket is padded to a
common slot budget so one SPMD program serves all cores.

Device pipeline per window:
  - 4x dma_gather (one per column chunk, int16 chunk-relative indices) pull
    the 512B embedding rows for up to `budget` edges each into SBUF.
  - per 128-edge subtile, one fused DVE tensor_scalar builds the val-weighted
    one-hot S[e, r] = val[e] * (row_rel[e] == r) from a constant iota tile.
  - TensorE matmuls S^T @ G accumulate the window's [128,128] block in PSUM.
  - PSUM -> SBUF -> DRAM.
"""

import sys

import numpy as np

for _p in ("/opt/trn_rl_repo", "/root/problem"):
    if _p not in sys.path:
        sys.path.insert(0, _p)

N_NODES = 100000
D = 128
N_CORES = 8
B = N_NODES // N_CORES          # 12500 destination rows per core
WIN = 128                       # output window rows (= PSUM partition dim)
NW = (B + WIN - 1) // WIN       # 98 windows per core
B_PAD = NW * WIN                # 12544 padded rows per core
NCH = 4                         # column chunks (int16 index range)
CH = N_NODES // NCH             # 25000 rows per chunk

_cache = {}


def _build(budget, repeat=1):
    """Build + schedule the SPMD bass program for a per-(window,chunk) slot
    budget (multiple of 128). Returns the compiled Bacc module.

    repeat > 1 wraps the compute body in an on-device For_i loop — used only
    by the perf harness to amortize dispatch overhead when measuring."""
    import contextlib

    import concourse.mybir as mybir
    import concourse.tile as tile
    from concourse import bacc

    nsub_ch = budget // 128          # subtiles per chunk segment
    nsub = NCH * nsub_ch             # subtiles per window
    idx_cols = budget // 16          # idx16 columns per (window, chunk)

    nc = bacc.Bacc("TRN2", target_bir_lowering=False, debug=False,
                   num_devices=N_CORES, num_swdge_queues=4)
    embeds = nc.dram_tensor("embeds", [N_NODES, D], mybir.dt.float32,
                            kind="ExternalInput")
    idx16 = nc.dram_tensor("idx16", [128, NW * NCH * idx_cols], mybir.dt.int16,
                           kind="ExternalInput")
    rows_rel = nc.dram_tensor("rows_rel", [128, NW * nsub], mybir.dt.float32,
                              kind="ExternalInput")
    vals = nc.dram_tensor("vals", [128, NW * nsub], mybir.dt.float32,
                          kind="ExternalInput")
    out = nc.dram_tensor("out", [B_PAD, D], mybir.dt.float32,
                         kind="ExternalOutput")

    with tile.TileContext(nc) as tc:
        with (
            tc.tile_pool(name="const", bufs=1) as const_pool,
            tc.tile_pool(name="gather", bufs=3) as g_pool,
            tc.tile_pool(name="s", bufs=8) as s_pool,
            tc.tile_pool(name="o", bufs=2) as o_pool,
            tc.tile_pool(name="psum", bufs=6, space="PSUM") as psum_pool,
        ):
            iota_i = const_pool.tile([128, 128], mybir.dt.int32)
            nc.gpsimd.iota(iota_i[:], pattern=[[1, 128]], base=0,
                           channel_multiplier=0)
            iota_f = const_pool.tile([128, 128], mybir.dt.float32)
            nc.vector.tensor_copy(out=iota_f[:], in_=iota_i[:])

            idx_all = const_pool.tile([128, NW * NCH * idx_cols], mybir.dt.int16)
            nc.sync.dma_start(out=idx_all[:], in_=idx16[:])
            rows_all = const_pool.tile([128, NW * nsub], mybir.dt.float32)
            nc.sync.dma_start(out=rows_all[:], in_=rows_rel[:])
            vals_all = const_pool.tile([128, NW * nsub], mybir.dt.float32)
            nc.sync.dma_start(out=vals_all[:], in_=vals[:])

            loop = tc.For_i(0, repeat, 1) if repeat > 1 else contextlib.nullcontext()
            with loop:
                for w in range(NW):
                    G = g_pool.tile([128, nsub * 128], mybir.dt.float32)
                    for c in range(NCH):
                        seg = G[:, c * budget : (c + 1) * budget]
                        ioff = (w * NCH + c) * idx_cols
                        nc.gpsimd.dma_gather(
                            out_ap=seg.rearrange("p (j d) -> p j d", d=128),
                            in_ap=embeds[c * CH : (c + 1) * CH, :],
                            idxs_ap=idx_all[:, ioff : ioff + idx_cols],
                            num_idxs=budget,
                            num_idxs_reg=budget,
                            elem_size=D,
                            single_packet=False,
                            queue_num=c,
                        )
                    acc = psum_pool.tile([128, 128], mybir.dt.float32, space="PSUM")
                    for j in range(nsub):
                        col = w * nsub + j
                        S = s_pool.tile([128, 128], mybir.dt.float32, tag="S")
                        nc.vector.tensor_scalar(
                            out=S[:],
                            in0=iota_f[:],
                            scalar1=rows_all[:, col : col + 1],
                            scalar2=vals_all[:, col : col + 1],
                            op0=mybir.AluOpType.is_equal,
                            op1=mybir.AluOpType.mult,
                        )
                        nc.tensor.matmul(
                            out=acc[:],
                            lhsT=S[:],
                            rhs=G[:, j * 128 : (j + 1) * 128],
                            start=(j == 0),
                            stop=(j == nsub - 1),
                        )
                    o = o_pool.tile([128, 128], mybir.dt.float32)
                    nc.scalar.copy(out=o[:], in_=acc[:])
                    nc.sync.dma_start(out=out[w * 128 : (w + 1) * 128, :], in_=o[:])

    nc.compile()
    return nc


def _prep(edge_index, edge_vals):
    """Bucket + pad edges; returns (budget, per-core input dicts)."""
    rows = np.asarray(edge_index[0], dtype=np.int64)
    cols = np.asarray(edge_index[1], dtype=np.int64)
    vals = np.asarray(edge_vals, dtype=np.float32)
    E = rows.shape[0]

    core = rows // B
    row_local = rows - core * B
    w = row_local // WIN
    row_rel = (row_local - w * WIN).astype(np.float32)
    ch = cols // CH
    col_rel = (cols - ch * CH).astype(np.int16)

    bucket = ((core * NW + w) * NCH + ch).astype(np.int64)
    n_buckets = N_CORES * NW * NCH
    counts = np.bincount(bucket, minlength=n_buckets)
    budget = int(-(-counts.max() // 128) * 128)

    order = np.argsort(bucket, kind="stable")
    starts = np.zeros(n_buckets, dtype=np.int64)
    np.cumsum(counts[:-1], out=starts[1:])
    pos = np.arange(E, dtype=np.int64) - starts[bucket[order]]

    bo = bucket[order]
    slot = bo * budget + pos            # global slot id across all cores

    n_slots = n_buckets * budget
    idx_lin = np.zeros(n_slots, dtype=np.int16)
    rows_lin = np.zeros(n_slots, dtype=np.float32)
    vals_lin = np.zeros(n_slots, dtype=np.float32)
    idx_lin[slot] = col_rel[order]
    rows_lin[slot] = row_rel[order]
    vals_lin[slot] = vals[order]

    nsub_ch = budget // 128
    nsub = NCH * nsub_ch
    in_maps = []
    per_core = NW * NCH * budget
    for k in range(N_CORES):
        lin = slice(k * per_core, (k + 1) * per_core)
        # idx16: [NW, NCH, budget] -> per segment [16, budget//16], tiled x8
        a = idx_lin[lin].reshape(NW, NCH, budget // 16, 16)
        idx16 = np.ascontiguousarray(
            a.transpose(3, 0, 1, 2).reshape(16, -1))
        idx16 = np.tile(idx16, (8, 1))
        # rows/vals: [NW, NCH*budget] ; slot i -> (partition i%128, col i//128)
        r = rows_lin[lin].reshape(NW, nsub, 128)
        rows_t = np.ascontiguousarray(r.transpose(2, 0, 1).reshape(128, -1))
        v = vals_lin[lin].reshape(NW, nsub, 128)
        vals_t = np.ascontiguousarray(v.transpose(2, 0, 1).reshape(128, -1))
        in_maps.append({"idx16": idx16, "rows_rel": rows_t, "vals": vals_t})
    return budget, in_maps


def kernel(embeds, edge_index, edge_vals):
    from concourse.bass_utils import run_bass_kernel_spmd

    embeds = np.ascontiguousarray(np.asarray(embeds, dtype=np.float32))
    budget, in_maps = _prep(edge_index, edge_vals)
    for m in in_maps:
        m["embeds"] = embeds

    if budget not in _cache:
        _cache[budget] = _build(budget)
    nc = _cache[budget]

    res = run_bass_kernel_spmd(nc, in_maps, core_ids=list(range(N_CORES)))
    out = np.empty((N_NODES, D), dtype=np.float32)
    for k in range(N_CORES):
        out[k * B : (k + 1) * B] = res.results[k]["out"][:B]
    return out

